# revision 1
# baseline (speedup 1.0000x reference)
"""Trainium2 Bass kernel for ControlLoRACrossAttnProcessor.

Head-parallel sharding over 8 NeuronCores: core c owns attention head c
(columns c*128:(c+1)*128 of Wq/Wk/Wv, rows of the same range in Wo's
contraction dim). Each core computes its head's full attention and a
partial output projection; the host sums the 8 partials. The rank-4
LoRA path is sharded by sequence rows (core c owns rows c*512:(c+1)*512)
and returned as a separate small output that the host adds in, together
with the output bias (added exactly once per row).

All matmuls run as float32r (TF32-like PE mode, full rate at moving
dim >= 256) with fp32 accumulation in PSUM. Attention uses the
transposed-scores layout: scores^T tiles [k=128, q=512] so softmax's
exp rides ScalarE and the k-sums ride TensorE (ones-vector matmul);
normalization is deferred to the output projection (divide commutes
with the linear Wo projection).
"""

import sys
import types

for _p in ("/opt/trn_rl_repo", "/root/.axon_site"):
    if _p not in sys.path:
        sys.path.insert(0, _p)

import numpy as np

import concourse.bass as bass  # noqa: E402
import concourse.mybir as mybir  # noqa: E402
from concourse import bacc  # noqa: E402
from concourse.bass_utils import run_bass_kernel_spmd  # noqa: E402
from concourse.tile import TileContext  # noqa: E402
from concourse.masks import make_identity  # noqa: E402

dt = mybir.dt

B, S, D = 2, 2048, 1024
H = 8
HD = 128
RANK = 4
N_CORES = 8
SG = B * S            # 4096 flattened rows
ROWS_PER_CORE = SG // N_CORES  # 512
NSTRIP = S // 512     # 4 query strips of 512 per batch
NKT = S // 128        # 16 key tiles of 128 per batch
NQT = 512 // 128      # 4 query tiles per strip
INV_SQRT_HD = 1.0 / np.sqrt(np.float32(HD))

F32 = dt.float32
F32R = dt.float32r

_CACHE = {}


def build_program():
    if "nc" in _CACHE:
        return _CACHE["nc"]

    nc = bacc.Bacc("TRN2", target_bir_lowering=False, debug=False,
                   num_devices=N_CORES)

    xT = nc.declare_dram_parameter("xT", [D, SG], F32R, isOutput=False)
    wqT = nc.declare_dram_parameter("wqT", [D, HD], F32R, isOutput=False)
    wkT = nc.declare_dram_parameter("wkT", [D, HD], F32R, isOutput=False)
    wvT = nc.declare_dram_parameter("wvT", [D, HD], F32R, isOutput=False)
    woT = nc.declare_dram_parameter("woT", [HD, D], F32R, isOutput=False)
    cT = nc.declare_dram_parameter("cT", [D, ROWS_PER_CORE], F32R, isOutput=False)
    ldT = nc.declare_dram_parameter("ldT", [D, RANK], F32R, isOutput=False)
    luT = nc.declare_dram_parameter("luT", [RANK, D], F32R, isOutput=False)
    bo = nc.declare_dram_parameter("bo", [1, D], F32, isOutput=False)
    out = nc.declare_dram_parameter("out", [SG, D], F32, isOutput=True)
    lora_out = nc.declare_dram_parameter("lora_out", [ROWS_PER_CORE, D], F32,
                                         isOutput=True)

    with TileContext(nc) as tc:
        with tc.tile_pool(name="const", bufs=1) as constp, \
             tc.tile_pool(name="wts", bufs=1) as wts, \
             tc.tile_pool(name="op_ps", bufs=2, space="PSUM") as op_ps, \
             tc.tile_pool(name="sc_ps", bufs=2, space="PSUM") as sc_ps, \
             tc.tile_pool(name="at_ps", bufs=1, space="PSUM") as at_ps, \
             tc.tile_pool(name="sum_ps", bufs=1, space="PSUM") as sum_ps, \
             tc.tile_pool(name="xt", bufs=2) as xtp, \
             tc.tile_pool(name="qkv", bufs=2) as qkvp, \
             tc.tile_pool(name="es", bufs=5) as esp, \
             tc.tile_pool(name="small", bufs=2) as smallp, \
             tc.tile_pool(name="outp", bufs=2) as outp, \
             tc.tile_pool(name="ct", bufs=1) as ctp:

            # ---- constants & weight loads (smallest first) ----
            ident = constp.tile([128, 128], F32, tag="ident")
            make_identity(nc, ident[:])
            ones_f = constp.tile([128, 1], F32, tag="onesf")
            nc.vector.memset(ones_f[:], 1.0)
            ones = constp.tile([128, 1], F32R, tag="ones")
            nc.vector.tensor_copy(ones[:], ones_f[:])

            wq_sb = wts.tile([128, D], F32R, tag="wq")
            wk_sb = wts.tile([128, D], F32R, tag="wk")
            wv_sb = wts.tile([128, D], F32R, tag="wv")
            wo_sb = wts.tile([HD, D], F32R, tag="wo")
            lu_sb = wts.tile([RANK, D], F32R, tag="lu")
            ld_sb = wts.tile([128, 8 * RANK], F32R, tag="ld")
            bo_sb = wts.tile([1, D], F32, tag="bo")
            bo_bc = wts.tile([128, D], F32, tag="bobc")
            for _w_sb, _wT in ((wq_sb, wqT), (wk_sb, wkT), (wv_sb, wvT)):
                nc.sync.dma_start(
                    out=_w_sb[:].rearrange("p (t m) -> p t m", t=8),
                    in_=_wT[:].rearrange("(t p) m -> p t m", p=128))

            xt_tiles = {}

            def load_xt(b):
                tiles = [xtp.tile([128, 4 * S], F32R, tag="xt",
                                  name=f"xt{b}_{h}") for h in range(2)]
                for h in range(2):
                    nc.sync.dma_start(
                        out=tiles[h][:].rearrange("p (t s) -> p t s", t=4),
                        in_=xT[h * 512:(h + 1) * 512, b * S:(b + 1) * S]
                        .rearrange("(t p) s -> p t s", p=128))
                xt_tiles[b] = tiles

            load_xt(0)

            ct_sb = ctp.tile([128, 8 * ROWS_PER_CORE], F32R, tag="ct")
            nc.sync.dma_start(
                out=ct_sb[:].rearrange("p (t m) -> p t m", t=8),
                in_=cT[:].rearrange("(t p) m -> p t m", p=128))
            nc.sync.dma_start(out=wo_sb[:], in_=woT[:])
            nc.sync.dma_start(out=lu_sb[:], in_=luT[:])
            nc.sync.dma_start(out=ld_sb[:].rearrange("p (t m) -> p t m", t=8),
                              in_=ldT[:].rearrange("(t p) m -> p t m", p=128))
            nc.sync.dma_start(out=bo_sb[:], in_=bo[:])
            nc.gpsimd.partition_broadcast(bo_bc[:], bo_sb[:])

            # PE warmup while the first activation DMAs land: keeps the HAM
            # clock-gate warm and fills the otherwise-idle load window.
            wu_ps = sum_ps.tile([1, 512], F32, tag="sums")
            for _wu in range(48):
                nc.tensor.matmul(wu_ps[:], ones[:], wq_sb[:, 0:512],
                                 start=True, stop=True)

            def emit_lora():
                # rows [c*512, (c+1)*512) of up(down(ctrl)) + bias
                dn_ps = op_ps.tile([RANK, ROWS_PER_CORE], F32, tag="op",
                                   name="dn_ps")
                for d in range(8):
                    nc.tensor.matmul(
                        dn_ps[:],
                        ld_sb[:, d * RANK:(d + 1) * RANK],
                        ct_sb[:, d * ROWS_PER_CORE:(d + 1) * ROWS_PER_CORE],
                        start=(d == 0), stop=(d == 7))
                dn_sb = smallp.tile([RANK, ROWS_PER_CORE], F32R, tag="dn")
                nc.vector.tensor_copy(dn_sb[:], dn_ps[:])
                for j in range(ROWS_PER_CORE // 128):
                    lo_sb = outp.tile([128, D], F32, tag="osb", name="lo_sb")
                    for g in range(2):
                        up_ps = op_ps.tile([128, 512], F32, tag="op",
                                           name="up_ps")
                        nc.tensor.matmul(
                            up_ps[:],
                            dn_sb[:, j * 128:(j + 1) * 128],
                            lu_sb[:, g * 512:(g + 1) * 512],
                            start=True, stop=True)
                        nc.vector.tensor_add(
                            lo_sb[:, g * 512:(g + 1) * 512], up_ps[:],
                            bo_bc[:, g * 512:(g + 1) * 512])
                    nc.sync.dma_start(out=lora_out[j * 128:(j + 1) * 128, :],
                                      in_=lo_sb[:])

            def emit_qkv(b):
                if b not in xt_tiles:
                    load_xt(b)
                xt = xt_tiles[b]

                qt_sb = qkvp.tile([HD, S], F32R, tag="qt", name=f"qt{b}")
                kt_sb = qkvp.tile([HD, S], F32R, tag="kt", name=f"kt{b}")
                vt_sb = qkvp.tile([HD, S], F32, tag="vt", bufs=1,
                                  name=f"vt{b}")
                v_sb = qkvp.tile([128, S], F32R, tag="v", name=f"v{b}")

                def proj_half(w_sb, dst, strip, h):
                    # contraction split in two PSUM groups per strip so the
                    # h=0 half can run while the second xT half still loads
                    ps2 = sc_ps.tile([128, 1024], F32, tag="sc", name="ps2")
                    ps = ps2[:, 0:512]
                    for dl in range(4):
                        d = h * 4 + dl
                        nc.tensor.matmul(
                            ps,
                            w_sb[:, d * HD:(d + 1) * HD],
                            xt[h][:, dl * S + strip * 512:
                                    dl * S + strip * 512 + 512],
                            start=(dl == 0), stop=(dl == 3),
                            skip_group_check=True)
                    sl = slice(strip * 512, (strip + 1) * 512)
                    if h == 0:
                        nc.vector.tensor_copy(dst[:, sl], ps)
                    else:
                        nc.vector.tensor_add(dst[:, sl], ps, dst[:, sl])

                for strip in range(NSTRIP):
                    proj_half(wv_sb, vt_sb, strip, 0)
                for strip in range(NSTRIP):
                    proj_half(wq_sb, qt_sb, strip, 0)
                for strip in range(NSTRIP):
                    proj_half(wk_sb, kt_sb, strip, 0)
                # second halves; V transposes interleave to keep PE dense
                for strip in range(NSTRIP):
                    proj_half(wv_sb, vt_sb, strip, 1)
                for strip in range(NSTRIP):
                    proj_half(wq_sb, qt_sb, strip, 1)
                    tq2 = sc_ps.tile([128, 1024], F32, tag="sc", name="tq2")
                    for i, kt in enumerate(range(4 * strip, 4 * strip + 4)):
                        nc.tensor.transpose(
                            tq2[:, i * 128:(i + 1) * 128],
                            vt_sb[:, kt * 128:(kt + 1) * 128], ident[:])
                    nc.vector.tensor_copy(
                        v_sb[:, strip * 512:(strip + 1) * 512],
                        tq2[:, 0:512])
                for strip in range(NSTRIP):
                    proj_half(wk_sb, kt_sb, strip, 1)
                return qt_sb, kt_sb, v_sb

            def emit_attention(b, qt_sb, kt_sb, v_sb):
                for strip in range(NSTRIP):
                    q_sl = slice(strip * 512, (strip + 1) * 512)
                    at_ps_t = at_ps.tile([HD, 512], F32, tag="at",
                                         name="at_ps_t")
                    sm_ps = sum_ps.tile([1, 512], F32, tag="sums",
                                        name="sm_ps")
                    for p in range(NKT // 2):
                        kt0 = 2 * p
                        scp = sc_ps.tile([128, 1024], F32, tag="sc",
                                         name="scp")
                        for i in range(2):
                            nc.tensor.matmul(
                                scp[:, i * 512:(i + 1) * 512],
                                kt_sb[:, (kt0 + i) * 128:(kt0 + i + 1) * 128],
                                qt_sb[:, q_sl],
                                start=True, stop=True,
                                skip_group_check=True)
                        es2 = esp.tile([128, 1024], F32R, tag="es",
                                       name="es2")
                        nc.scalar.activation(
                            es2[:], scp[:], mybir.ActivationFunctionType.Exp,
                            scale=float(INV_SQRT_HD))
                        for i in range(2):
                            kt = kt0 + i
                            nc.tensor.matmul(
                                at_ps_t[:],
                                v_sb[:, kt * 128:(kt + 1) * 128],
                                es2[:, i * 512:(i + 1) * 512],
                                start=(kt == 0), stop=(kt == NKT - 1),
                                skip_group_check=True)
                            nc.tensor.matmul(
                                sm_ps[:],
                                ones[:],
                                es2[:, i * 512:(i + 1) * 512],
                                start=(kt == 0), stop=(kt == NKT - 1),
                                skip_group_check=True)

                    # sums [1,512] -> SBUF row -> scatter to [128,4] columns
                    # -> 128-lane reciprocal (a [1,512] reciprocal would run
                    # serially on one DVE lane, ~3.3us)
                    row_sm = smallp.tile([1, 512], F32, tag="rowsm",
                                         name="row_sm")
                    nc.vector.tensor_copy(row_sm[:], sm_ps[:])
                    rcol_sb = smallp.tile([128, NQT], F32, tag="rcol",
                                          name="rcol_sb")
                    for j in range(NQT):
                        nc.sync.dma_start(
                            out=rcol_sb[:, j:j + 1],
                            in_=row_sm[0:1, j * 128:(j + 1) * 128])
                    rc_sb = smallp.tile([128, NQT], F32, tag="rc",
                                        name="rc_sb")
                    nc.vector.reciprocal(rc_sb[:], rcol_sb[:])

                    atn_sb = smallp.tile([HD, 512], F32R, tag="atn",
                                         name="atn_sb")
                    nc.vector.tensor_copy(atn_sb[:], at_ps_t[:])

                    # output projection + deferred softmax normalization
                    for j in range(NQT):
                        o_sb = outp.tile([128, D], F32, tag="osb", name="o_sb")
                        for g in range(2):
                            op = op_ps.tile([128, 512], F32, tag="op",
                                            name="op")
                            nc.tensor.matmul(
                                op[:],
                                atn_sb[:, j * 128:(j + 1) * 128],
                                wo_sb[:, g * 512:(g + 1) * 512],
                                start=True, stop=True)
                            nc.vector.tensor_scalar_mul(
                                o_sb[:, g * 512:(g + 1) * 512], op[:],
                                rc_sb[:, j:j + 1])
                        r0 = b * S + strip * 512 + j * 128
                        nc.sync.dma_start(out=out[r0:r0 + 128, :], in_=o_sb[:])

            qkv0 = emit_qkv(0)
            emit_lora()
            qkv1 = emit_qkv(1)
            emit_attention(0, *qkv0)
            emit_attention(1, *qkv1)

    nc.compile()
    _CACHE["nc"] = nc
    return nc


def _prep_in_maps(inputs):
    hidden = np.ascontiguousarray(inputs["hidden_states"], dtype=np.float32)
    control = np.ascontiguousarray(inputs["control_states"], dtype=np.float32)
    Wq = np.asarray(inputs["Wq"], dtype=np.float32)
    Wk = np.asarray(inputs["Wk"], dtype=np.float32)
    Wv = np.asarray(inputs["Wv"], dtype=np.float32)
    Wo = np.asarray(inputs["Wo"], dtype=np.float32)
    bo = np.asarray(inputs["bo"], dtype=np.float32)
    ld = np.asarray(inputs["lora_down"], dtype=np.float32)
    lu = np.asarray(inputs["lora_up"], dtype=np.float32)

    xT = np.ascontiguousarray(hidden.reshape(SG, D).T)
    cT_full = np.ascontiguousarray(control.reshape(SG, D).T)
    ldT = np.ascontiguousarray(ld.T)
    luT = np.ascontiguousarray(lu.T)
    bo_in = np.ascontiguousarray(bo.reshape(1, D))

    in_maps = []
    for c in range(N_CORES):
        hs = slice(c * HD, (c + 1) * HD)
        rs = slice(c * ROWS_PER_CORE, (c + 1) * ROWS_PER_CORE)
        in_maps.append({
            "xT": xT,
            "wqT": np.ascontiguousarray(Wq[hs, :].T),
            "wkT": np.ascontiguousarray(Wk[hs, :].T),
            "wvT": np.ascontiguousarray(Wv[hs, :].T),
            "woT": np.ascontiguousarray(Wo[:, hs].T),
            "cT": np.ascontiguousarray(cT_full[:, rs]),
            "ldT": ldT,
            "luT": luT,
            "bo": bo_in,
        })
    return in_maps


def _reduce_outputs(results):
    total = np.zeros((SG, D), dtype=np.float64)
    for c in range(N_CORES):
        total += results[c]["out"].astype(np.float64)
    total = total.astype(np.float32)
    for c in range(N_CORES):
        rs = slice(c * ROWS_PER_CORE, (c + 1) * ROWS_PER_CORE)
        total[rs] += results[c]["lora_out"]
    return total.reshape(B, S, D)


def kernel(**inputs):
    nc = build_program()
    in_maps = _prep_in_maps(inputs)
    res = run_bass_kernel_spmd(nc, in_maps, list(range(N_CORES)))
    return _reduce_outputs(res.results)



# revision 2
# speedup vs baseline: 1.5015x; 1.5015x over previous
"""Trainium2 Bass kernel for ControlLoRACrossAttnProcessor.

Head-parallel sharding over 8 NeuronCores: core c owns attention head c
(columns c*128:(c+1)*128 of Wq/Wk/Wv, rows of the same range in Wo's
contraction dim). Each core computes its head's full attention and a
partial output projection; the host sums the 8 partials. The rank-4
LoRA path is sharded by sequence rows (core c owns rows c*512:(c+1)*512)
and returned as a separate small output that the host adds in, together
with the output bias (added exactly once per row).

All matmuls run in bf16 (1 cycle/row on the PE vs 2 for f32r) with fp32
accumulation in PSUM. Attention uses the transposed-scores layout:
scores^T tiles [k=128, q=512] so softmax's exp rides ScalarE; the
softmax denominators are built by summing exp tiles 4-at-a-time on the
DVE and row-summing the group totals with a ones-vector matmul
(quartering that matmul's PE cost); normalization is deferred to the
output projection (divide commutes with the linear Wo projection).
"""

import sys

for _p in ("/opt/trn_rl_repo", "/root/.axon_site"):
    if _p not in sys.path:
        sys.path.insert(0, _p)

import ml_dtypes
import numpy as np

import concourse.bass as bass  # noqa: E402
import concourse.mybir as mybir  # noqa: E402
from concourse import bacc  # noqa: E402
from concourse.bass_utils import run_bass_kernel_spmd  # noqa: E402
from concourse.tile import TileContext  # noqa: E402
from concourse.masks import make_identity  # noqa: E402

dt = mybir.dt

B, S, D = 2, 2048, 1024
H = 8
HD = 128
RANK = 4
N_CORES = 8
SG = B * S            # 4096 flattened rows
ROWS_PER_CORE = SG // N_CORES  # 512
NSTRIP = S // 512     # 4 query strips of 512 per batch
NKT = S // 128        # 16 key tiles of 128 per batch
NQT = 512 // 128      # 4 query tiles per strip
INV_SQRT_HD = 1.0 / np.sqrt(np.float32(HD))

F32 = dt.float32
BF = dt.bfloat16
NPBF = ml_dtypes.bfloat16

_CACHE = {}


def build_program():
    if "nc" in _CACHE:
        return _CACHE["nc"]

    nc = bacc.Bacc("TRN2", target_bir_lowering=False, debug=False,
                   num_devices=N_CORES)

    xT = nc.declare_dram_parameter("xT", [D, SG], BF, isOutput=False)
    wqT = nc.declare_dram_parameter("wqT", [D, HD], BF, isOutput=False)
    wkT = nc.declare_dram_parameter("wkT", [D, HD], BF, isOutput=False)
    wvT = nc.declare_dram_parameter("wvT", [D, HD], BF, isOutput=False)
    woT = nc.declare_dram_parameter("woT", [HD, D], BF, isOutput=False)
    cT = nc.declare_dram_parameter("cT", [D, ROWS_PER_CORE], BF, isOutput=False)
    ldT = nc.declare_dram_parameter("ldT", [D, RANK], BF, isOutput=False)
    luT = nc.declare_dram_parameter("luT", [RANK, D], BF, isOutput=False)
    bo = nc.declare_dram_parameter("bo", [1, D], F32, isOutput=False)
    out = nc.declare_dram_parameter("out", [SG, D], BF, isOutput=True)
    lora_out = nc.declare_dram_parameter("lora_out", [ROWS_PER_CORE, D], BF,
                                         isOutput=True)

    with TileContext(nc) as tc:
        with tc.tile_pool(name="const", bufs=1) as constp, \
             tc.tile_pool(name="wts", bufs=1) as wts, \
             tc.tile_pool(name="op_ps", bufs=2, space="PSUM") as op_ps, \
             tc.tile_pool(name="sc_ps", bufs=2, space="PSUM") as sc_ps, \
             tc.tile_pool(name="at_ps", bufs=1, space="PSUM") as at_ps, \
             tc.tile_pool(name="sum_ps", bufs=1, space="PSUM") as sum_ps, \
             tc.tile_pool(name="xt", bufs=2) as xtp, \
             tc.tile_pool(name="qkv", bufs=2) as qkvp, \
             tc.tile_pool(name="es", bufs=5) as esp, \
             tc.tile_pool(name="esum", bufs=3) as esump, \
             tc.tile_pool(name="small", bufs=2) as smallp, \
             tc.tile_pool(name="outp", bufs=2) as outp, \
             tc.tile_pool(name="ct", bufs=1) as ctp:

            # ---- constants (no DMA deps: warmup can start immediately) ----
            ident = constp.tile([128, 128], BF, tag="ident")
            make_identity(nc, ident[:])
            ones_f = constp.tile([128, 1], F32, tag="onesf")
            nc.vector.memset(ones_f[:], 1.0)
            ones = constp.tile([128, 1], BF, tag="ones")
            nc.vector.tensor_copy(ones[:], ones_f[:])
            wfill = constp.tile([128, 512], BF, tag="wfill")
            nc.vector.memset(wfill[:], 0.5)

            # PE warmup while the weight/activation DMAs land: keeps the
            # PE p-state ramped and fills the otherwise-idle load window.
            wu_ps = sum_ps.tile([1, 512], F32, tag="sums")
            for _wu in range(44):
                nc.tensor.matmul(wu_ps[:], ones[:], wfill[:],
                                 start=True, stop=True)

            wq_sb = wts.tile([128, D], BF, tag="wq")
            wk_sb = wts.tile([128, D], BF, tag="wk")
            wv_sb = wts.tile([128, D], BF, tag="wv")
            wo_sb = wts.tile([HD, D], BF, tag="wo")
            lu_sb = wts.tile([RANK, D], BF, tag="lu")
            ld_sb = wts.tile([128, 8 * RANK], BF, tag="ld")
            bo_sb = wts.tile([1, D], F32, tag="bo")
            bo_bc = wts.tile([128, D], F32, tag="bobc")
            for _w_sb, _wT in ((wv_sb, wvT), (wq_sb, wqT), (wk_sb, wkT)):
                nc.sync.dma_start(
                    out=_w_sb[:].rearrange("p (t m) -> p t m", t=8),
                    in_=_wT[:].rearrange("(t p) m -> p t m", p=128))

            xt_tiles = {}

            def load_xt(b):
                tiles = [xtp.tile([128, 4 * S], BF, tag="xt",
                                  name=f"xt{b}_{h}") for h in range(2)]
                for h in range(2):
                    nc.sync.dma_start(
                        out=tiles[h][:].rearrange("p (t s) -> p t s", t=4),
                        in_=xT[h * 512:(h + 1) * 512, b * S:(b + 1) * S]
                        .rearrange("(t p) s -> p t s", p=128))
                xt_tiles[b] = tiles

            load_xt(0)

            ct_sb = ctp.tile([128, 8 * ROWS_PER_CORE], BF, tag="ct")
            nc.sync.dma_start(
                out=ct_sb[:].rearrange("p (t m) -> p t m", t=8),
                in_=cT[:].rearrange("(t p) m -> p t m", p=128))
            nc.sync.dma_start(out=wo_sb[:], in_=woT[:])
            nc.sync.dma_start(out=lu_sb[:], in_=luT[:])
            nc.sync.dma_start(out=ld_sb[:].rearrange("p (t m) -> p t m", t=8),
                              in_=ldT[:].rearrange("(t p) m -> p t m", p=128))
            nc.sync.dma_start(out=bo_sb[:], in_=bo[:])
            nc.gpsimd.partition_broadcast(bo_bc[:], bo_sb[:])

            def emit_lora():
                # rows [c*512, (c+1)*512) of up(down(ctrl)) + bias
                dn_ps = op_ps.tile([RANK, ROWS_PER_CORE], F32, tag="op",
                                   name="dn_ps")
                for d in range(8):
                    nc.tensor.matmul(
                        dn_ps[:],
                        ld_sb[:, d * RANK:(d + 1) * RANK],
                        ct_sb[:, d * ROWS_PER_CORE:(d + 1) * ROWS_PER_CORE],
                        start=(d == 0), stop=(d == 7))
                dn_sb = smallp.tile([RANK, ROWS_PER_CORE], BF, tag="dn")
                nc.vector.tensor_copy(dn_sb[:], dn_ps[:])
                for j in range(ROWS_PER_CORE // 128):
                    lo_sb = outp.tile([128, D], BF, tag="osb", name="lo_sb")
                    for g in range(2):
                        up_ps = op_ps.tile([128, 512], F32, tag="op",
                                           name="up_ps")
                        nc.tensor.matmul(
                            up_ps[:],
                            dn_sb[:, j * 128:(j + 1) * 128],
                            lu_sb[:, g * 512:(g + 1) * 512],
                            start=True, stop=True)
                        nc.vector.tensor_add(
                            lo_sb[:, g * 512:(g + 1) * 512], up_ps[:],
                            bo_bc[:, g * 512:(g + 1) * 512])
                    nc.sync.dma_start(out=lora_out[j * 128:(j + 1) * 128, :],
                                      in_=lo_sb[:])

            def emit_qkv(b):
                if b not in xt_tiles:
                    load_xt(b)
                xt = xt_tiles[b]

                qt_sb = qkvp.tile([HD, S], BF, tag="qt", name=f"qt{b}")
                kt_sb = qkvp.tile([HD, S], BF, tag="kt", name=f"kt{b}")
                vt_sb = qkvp.tile([HD, S], BF, tag="vt", bufs=1,
                                  name=f"vt{b}")
                v_sb = qkvp.tile([128, S], BF, tag="v", name=f"v{b}")

                def proj_half(w_sb, dst, strip, h):
                    # contraction split in two PSUM groups per strip so the
                    # h=0 half can run while the second xT half still loads
                    ps2 = sc_ps.tile([128, 1024], F32, tag="sc", name="ps2")
                    ps = ps2[:, 0:512]
                    for dl in range(4):
                        d = h * 4 + dl
                        nc.tensor.matmul(
                            ps,
                            w_sb[:, d * HD:(d + 1) * HD],
                            xt[h][:, dl * S + strip * 512:
                                    dl * S + strip * 512 + 512],
                            start=(dl == 0), stop=(dl == 3),
                            skip_group_check=True)
                    sl = slice(strip * 512, (strip + 1) * 512)
                    if h == 0:
                        nc.vector.tensor_copy(dst[:, sl], ps)
                    else:
                        nc.vector.tensor_add(dst[:, sl], ps, dst[:, sl])

                for strip in range(NSTRIP):
                    proj_half(wv_sb, vt_sb, strip, 0)
                for strip in range(NSTRIP):
                    proj_half(wq_sb, qt_sb, strip, 0)
                for strip in range(NSTRIP):
                    proj_half(wk_sb, kt_sb, strip, 0)
                # second halves; V transposes interleave to keep PE dense
                for strip in range(NSTRIP):
                    proj_half(wv_sb, vt_sb, strip, 1)
                for strip in range(NSTRIP):
                    proj_half(wq_sb, qt_sb, strip, 1)
                    tq2 = sc_ps.tile([128, 1024], BF, tag="sc", name="tq2")
                    for i, kt in enumerate(range(4 * strip, 4 * strip + 4)):
                        nc.tensor.transpose(
                            tq2[:, i * 128:(i + 1) * 128],
                            vt_sb[:, kt * 128:(kt + 1) * 128], ident[:])
                    nc.vector.tensor_copy(
                        v_sb[:, strip * 512:(strip + 1) * 512],
                        tq2[:, 0:512])
                for strip in range(NSTRIP):
                    proj_half(wk_sb, kt_sb, strip, 1)
                return qt_sb, kt_sb, v_sb

            def emit_attention(b, qt_sb, kt_sb, v_sb):
                for strip in range(NSTRIP):
                    q_sl = slice(strip * 512, (strip + 1) * 512)
                    at_ps_t = at_ps.tile([HD, 512], F32, tag="at",
                                         name="at_ps_t")
                    sm_ps = sum_ps.tile([1, 512], F32, tag="sums",
                                        name="sm_ps")
                    part = [None, None]
                    for p in range(NKT // 2):
                        kt0 = 2 * p
                        scp = sc_ps.tile([128, 1024], F32, tag="sc",
                                         name="scp")
                        for i in range(2):
                            nc.tensor.matmul(
                                scp[:, i * 512:(i + 1) * 512],
                                kt_sb[:, (kt0 + i) * 128:(kt0 + i + 1) * 128],
                                qt_sb[:, q_sl],
                                start=True, stop=True,
                                skip_group_check=True)
                        es2 = esp.tile([128, 1024], BF, tag="es",
                                       name="es2")
                        nc.scalar.activation(
                            es2[:], scp[:], mybir.ActivationFunctionType.Exp,
                            scale=float(INV_SQRT_HD))
                        for i in range(2):
                            kt = kt0 + i
                            nc.tensor.matmul(
                                at_ps_t[:],
                                v_sb[:, kt * 128:(kt + 1) * 128],
                                es2[:, i * 512:(i + 1) * 512],
                                start=(kt == 0), stop=(kt == NKT - 1),
                                skip_group_check=True)
                        # 4-way exp-sum tree on DVE: quarters the PE cost of
                        # the ones-vector denominator matmuls
                        h = p % 2
                        ptile = esump.tile([128, 512], BF, tag="et",
                                           name=f"pt{h}")
                        nc.vector.tensor_add(ptile[:], es2[:, 0:512],
                                             es2[:, 512:1024])
                        part[h] = ptile
                        if h == 1:
                            gtile = esump.tile([128, 512], BF, tag="et",
                                               name="gt")
                            nc.vector.tensor_add(gtile[:], part[0][:],
                                                 part[1][:])
                            nc.tensor.matmul(
                                sm_ps[:], ones[:], gtile[:],
                                start=(p == 1), stop=(p == NKT // 2 - 1),
                                skip_group_check=True)

                    # sums [1,512] -> SBUF row -> scatter to [128,4] columns
                    # -> 128-lane reciprocal (a [1,512] reciprocal would run
                    # serially on one DVE lane, ~3.3us)
                    row_sm = smallp.tile([1, 512], F32, tag="rowsm",
                                         name="row_sm")
                    nc.vector.tensor_copy(row_sm[:], sm_ps[:])
                    rcol_sb = smallp.tile([128, NQT], F32, tag="rcol",
                                          name="rcol_sb")
                    for j in range(NQT):
                        nc.sync.dma_start(
                            out=rcol_sb[:, j:j + 1],
                            in_=row_sm[0:1, j * 128:(j + 1) * 128])
                    rc_sb = smallp.tile([128, NQT], F32, tag="rc",
                                        name="rc_sb")
                    nc.vector.reciprocal(rc_sb[:], rcol_sb[:])

                    atn_sb = smallp.tile([HD, 512], BF, tag="atn",
                                         name="atn_sb")
                    nc.vector.tensor_copy(atn_sb[:], at_ps_t[:])

                    # output projection + deferred softmax normalization
                    for j in range(NQT):
                        o_sb = outp.tile([128, D], BF, tag="osb", name="o_sb")
                        for g in range(2):
                            op = op_ps.tile([128, 512], F32, tag="op",
                                            name="op")
                            nc.tensor.matmul(
                                op[:],
                                atn_sb[:, j * 128:(j + 1) * 128],
                                wo_sb[:, g * 512:(g + 1) * 512],
                                start=True, stop=True)
                            nc.vector.tensor_scalar_mul(
                                o_sb[:, g * 512:(g + 1) * 512], op[:],
                                rc_sb[:, j:j + 1])
                        r0 = b * S + strip * 512 + j * 128
                        nc.sync.dma_start(out=out[r0:r0 + 128, :], in_=o_sb[:])

            qkv0 = emit_qkv(0)
            emit_lora()
            qkv1 = emit_qkv(1)
            emit_attention(0, *qkv0)
            emit_attention(1, *qkv1)

    nc.compile()
    _CACHE["nc"] = nc
    return nc


def _prep_in_maps(inputs):
    hidden = np.ascontiguousarray(inputs["hidden_states"], dtype=np.float32)
    control = np.ascontiguousarray(inputs["control_states"], dtype=np.float32)
    Wq = np.asarray(inputs["Wq"], dtype=np.float32)
    Wk = np.asarray(inputs["Wk"], dtype=np.float32)
    Wv = np.asarray(inputs["Wv"], dtype=np.float32)
    Wo = np.asarray(inputs["Wo"], dtype=np.float32)
    bo = np.asarray(inputs["bo"], dtype=np.float32)
    ld = np.asarray(inputs["lora_down"], dtype=np.float32)
    lu = np.asarray(inputs["lora_up"], dtype=np.float32)

    xT = np.ascontiguousarray(hidden.reshape(SG, D).T.astype(NPBF))
    cT_full = np.ascontiguousarray(control.reshape(SG, D).T.astype(NPBF))
    ldT = np.ascontiguousarray(ld.T.astype(NPBF))
    luT = np.ascontiguousarray(lu.T.astype(NPBF))
    bo_in = np.ascontiguousarray(bo.reshape(1, D))

    in_maps = []
    for c in range(N_CORES):
        hs = slice(c * HD, (c + 1) * HD)
        rs = slice(c * ROWS_PER_CORE, (c + 1) * ROWS_PER_CORE)
        in_maps.append({
            "xT": xT,
            "wqT": np.ascontiguousarray(Wq[hs, :].T.astype(NPBF)),
            "wkT": np.ascontiguousarray(Wk[hs, :].T.astype(NPBF)),
            "wvT": np.ascontiguousarray(Wv[hs, :].T.astype(NPBF)),
            "woT": np.ascontiguousarray(Wo[:, hs].T.astype(NPBF)),
            "cT": np.ascontiguousarray(cT_full[:, rs]),
            "ldT": ldT,
            "luT": luT,
            "bo": bo_in,
        })
    return in_maps


def _reduce_outputs(results):
    total = np.zeros((SG, D), dtype=np.float32)
    for c in range(N_CORES):
        total += results[c]["out"].astype(np.float32)
    for c in range(N_CORES):
        rs = slice(c * ROWS_PER_CORE, (c + 1) * ROWS_PER_CORE)
        total[rs] += results[c]["lora_out"].astype(np.float32)
    return total.reshape(B, S, D)


def kernel(**inputs):
    nc = build_program()
    in_maps = _prep_in_maps(inputs)
    res = run_bass_kernel_spmd(nc, in_maps, list(range(N_CORES)))
    return _reduce_outputs(res.results)
